# revision 15
# baseline (speedup 1.0000x reference)
"""ConvNeXt block kernel for Trainium2 (8 NeuronCores, data-parallel over batch).

Reference semantics (per image):
  y = x + gamma * ( GELU( LN(dwconv7x7(x) + dw_b) @ w1 + b1 ) @ w2 + b2 )
with LN over channels, exact (erf) GELU, NCHW in/out.

Distribution: batch 16 -> 2 images per core across 8 cores. No collectives.

v2 design: everything heavy runs on the tensor engine in fp8e4 with the
DoubleRow perf mode (2 K-tiles per pass at 0.5 cycles/row = 4x bf16).
 - depthwise conv: H+W zero-padded fp8 image (62x62 + 3-elem guards) kept
   contiguous per row, so every tap (d,e) is one contiguous 496-wide window
   per 8-row chunk.  49 taps -> 25 DoubleRow matmuls per (chunk, cb) against
   per-tap diagonal weights; pad columns land in PSUM and are discarded on
   evacuation (DVE, +dw_b bias, fp8 out).  dw weights are pre-scaled by 64
   (LN is scale-invariant, so no descale is needed anywhere).
 - LN stats: mu and E[y^2] accumulate into one [2,448] PSUM tile via fp8
   DoubleRow matmuls with {0,1}-column weights; rsqrt via the magic-constant
   Newton iteration in a [56,8] transposed layout (DRAM bounce), rstd
   broadcast to 128 partitions by a cast-DMA.
 - MLP: w1 pre-scaled by 16 with the LN mean-correction folded in as a 4th
   K-tile (row0 = -sum_c w1q, moving row0 = mu*rstd); GELU(psum/16 + b1) on
   ACT straight to fp8.  w2 pre-scaled by 32 with b2*32 as a 13th K-tile
   against an all-ones moving row; the final DVE op computes
   (psum * gamma/32) + x with x re-read from DRAM in fp32, so the dominant
   output term never leaves fp32.
Issue order software-pipelines conv 2 chunks ahead of the LN/MLP stream so
the PE never starves while the rstd chain resolves.
"""

import sys

sys.path.insert(0, "/opt/trn_rl_repo")

import numpy as np
import ml_dtypes

import bass_rust
import concourse.bass as bass
import concourse.mybir as mybir
import concourse.tile as tile
from concourse.bass_utils import run_bass_kernel_spmd

F32 = mybir.dt.float32
BF16 = mybir.dt.bfloat16
FP8 = mybir.dt.float8e4
I32 = mybir.dt.int32
AF = mybir.ActivationFunctionType
ALU = mybir.AluOpType
DR = mybir.MatmulPerfMode.DoubleRow

N_CORES = 8
IMGS_PER_CORE = 2
C = 384
CB = 3          # channel blocks of 128
H = W = 56
PIX = H * W     # 3136
WP = 62         # padded row width
GUARD = 3       # leading/trailing guard elems so tap bases stay in-bounds
IMLEN = GUARD + WP * WP + GUARD  # 3850
CHUNK = 448     # real pixels per chunk (8 rows)
CHUNKP = 496    # padded pixels per chunk (8 rows x 62)
NCHUNK = 7
FD = 1536
NFC = 12        # hidden blocks of 128
S_DW = 64.0     # dw-weight prescale (LN is scale-invariant)
S_W1 = 16.0
S_W2 = 32.0
SQS = 1.0 / 16.0  # ysq = (acc/16)^2 so fp8 range fits; var uses 256*msq
EPS_S = S_DW * S_DW * 1e-6
MAGIC = 0x5F3759DF

_WAITSPLIT_N = [0]


def _split_waits(nc, max_waits=1):
    """This walrus build rejects instructions with more than one sync-wait
    command; hoist excess waits onto dedicated NoOps on the same engine."""
    for fn in nc.m.functions:
        for bb in fn.blocks:
            insts = bb.instructions
            idx = 0
            while idx < len(insts):
                ins = insts[idx]
                si = ins.sync_info
                if si is not None and len(si.on_wait) > max_waits:
                    waits = list(si.on_wait)
                    extra, keep = waits[:-max_waits], waits[-max_waits:]
                    nops = []
                    for w in extra:
                        _WAITSPLIT_N[0] += 1
                        nops.append(
                            mybir.InstNoOp(
                                name=f"I-wsplit-{_WAITSPLIT_N[0]}",
                                engine=ins.engine,
                                ins=[],
                                outs=[],
                                sync_info=bass_rust.SyncInfo(
                                    on_wait=[w], on_update=[]
                                ),
                            )
                        )
                    ins.sync_info = bass_rust.SyncInfo(
                        on_wait=keep, on_update=list(si.on_update)
                    )
                    insts[idx:idx] = nops
                    idx += len(nops)
                idx += 1


def _cap(ap, dims, offset=None):
    """Custom strided AP (overlapping dims allowed)."""
    c = ap.copy()
    c.ap = bass_rust.VecI64Pair(list(dims))
    if offset is not None:
        c.offset = offset
    return c


def _build_nc(split_waits=True):
    nc = bass.Bass(trn_type="TRN2", target_bir_lowering=False, debug=False)

    xs = nc.dram_tensor("xs", [IMGS_PER_CORE, C, H, W], F32, kind="ExternalInput")
    xp8 = nc.dram_tensor("xp8", [IMGS_PER_CORE, CB, 128, IMLEN], FP8, kind="ExternalInput")
    diag8 = nc.dram_tensor("diag8", [128, CB * 50 * 128], FP8, kind="ExternalInput")
    dwb64 = nc.dram_tensor("dwb64", [128, CB], F32, kind="ExternalInput")
    w1q8 = nc.dram_tensor("w1q8", [128, 4 * NFC * 128], FP8, kind="ExternalInput")
    b1p = nc.dram_tensor("b1p", [128, NFC], F32, kind="ExternalInput")
    w2q8 = nc.dram_tensor("w2q8", [128, 14 * C], FP8, kind="ExternalInput")
    g32 = nc.dram_tensor("g32", [128, CB], F32, kind="ExternalInput")
    ys = nc.dram_tensor("ys", [IMGS_PER_CORE, C, H, W], F32, kind="ExternalOutput")
    vscratch = nc.dram_tensor("vscratch", [IMGS_PER_CORE, PIX], F32, kind="Internal")
    rscratch = nc.dram_tensor("rscratch", [IMGS_PER_CORE, PIX], BF16, kind="Internal")

    xs3 = xs.ap().rearrange("i c h w -> i c (h w)")
    ys3 = ys.ap().rearrange("i c h w -> i c (h w)")

    with tile.TileContext(nc) as tc:
        with (
            tc.tile_pool(name="const", bufs=1) as constp,
            tc.tile_pool(name="xpad", bufs=1) as xpadp,
            tc.tile_pool(name="acc", bufs=1) as accp,
            tc.tile_pool(name="ysq", bufs=2) as ysqp,
            tc.tile_pool(name="xts", bufs=2) as xtsp,
            tc.tile_pool(name="rb", bufs=2) as rbp,
            tc.tile_pool(name="h", bufs=2) as hp,
            tc.tile_pool(name="small", bufs=2) as smallp,
            tc.tile_pool(name="outp", bufs=3) as outp,
            tc.tile_pool(name="pstap", bufs=2, space="PSUM") as pstapp,
            tc.tile_pool(name="psstat", bufs=1, space="PSUM") as psstatp,
            tc.tile_pool(name="ps1", bufs=2, space="PSUM") as ps1p,
            tc.tile_pool(name="ps2", bufs=2, space="PSUM") as ps2p,
        ):
            # ---- static weights / constants ----
            dwb_sb = constp.tile([128, CB], F32)
            nc.sync.dma_start(dwb_sb[:], dwb64.ap())
            w1_sb = constp.tile([128, 4, NFC, 128], FP8)
            nc.sync.dma_start(
                w1_sb[:], w1q8.ap().rearrange("p (k f m) -> p k f m", k=4, f=NFC)
            )
            b1_sb = constp.tile([128, NFC], F32)
            nc.sync.dma_start(b1_sb[:], b1p.ap())
            w2_sb = constp.tile([128, 14, C], FP8)
            nc.sync.dma_start(w2_sb[:], w2q8.ap().rearrange("p (k c) -> p k c", k=14))
            g32_sb = constp.tile([128, CB], F32)
            nc.sync.dma_start(g32_sb[:], g32.ap())
            ones8 = constp.tile([128, CHUNK], FP8)
            nc.gpsimd.memset(ones8[:], 1.0)
            # stats matmul weights: [128, 2(kt), 1] ones selectors
            mw_full = constp.tile([128, 2, 32], FP8)
            nc.gpsimd.memset(mw_full[:], 0.0)
            nc.gpsimd.memset(mw_full[:, :, 0:1], 1.0)
            mw_half = constp.tile([128, 2, 32], FP8)
            nc.gpsimd.memset(mw_half[:], 0.0)
            nc.gpsimd.memset(mw_half[:, 0:1, 0:1], 1.0)

            # per-cb diagonal tap weights [128, 50, 128] (tap 49 = zeros),
            # precomputed on the host
            diags = []
            for cb in range(CB):
                diag = constp.tile([128, 50, 128], FP8, tag=f"diag{cb}")
                nc.sync.dma_start(
                    diag[:],
                    diag8.ap().rearrange("p (cb t m) -> p cb t m", cb=CB, t=50)[
                        :, cb
                    ],
                )
                diags.append(diag)

            xpads = {}
            acc_tiles = {}

            def emit_xpad(img):
                for cb in range(CB):
                    xp = xpadp.tile([128, IMLEN], FP8, tag=f"xp{img}{cb}")
                    nc.sync.dma_start(xp[:], xp8.ap()[img, cb])
                    xpads[(img, cb)] = xp

            def get_acc(img):
                if img not in acc_tiles:
                    acc_t = accp.tile([128, CB, PIX], FP8, tag=f"acc{img}")
                    acc_tiles[img] = acc_t
                return acc_tiles[img]

            def emit_conv_chunk(img, ch):
                h0 = ch * 8
                acc8 = get_acc(img)
                sl = slice(ch * CHUNK, (ch + 1) * CHUNK)
                for cb in range(CB):
                    xp = xpads[(img, cb)]
                    diag = diags[cb]
                    ps = pstapp.tile([128, CHUNKP], F32, tag="pstap", padded_shape=[128, 512])
                    # 21 within-row pairs + 3 cross-row pairs (e=3) + 1 single
                    mms = []
                    for d in range(-3, 4):
                        for e0 in (-3, -1, 1):
                            t = (d + 3) * 7 + (e0 + 3)
                            mms.append((
                                _cap(diag[:], [(50 * 128, 128), (128, 2), (1, 128)],
                                     offset=t * 128),
                                _cap(xp[:], [(IMLEN, 128), (1, 2), (1, CHUNKP)],
                                     offset=GUARD + (h0 + 3 + d) * WP + e0),
                            ))
                    for d0 in (-3, -1, 1):
                        t = (d0 + 3) * 7 + 6
                        mms.append((
                            _cap(diag[:], [(50 * 128, 128), (7 * 128, 2), (1, 128)],
                                 offset=t * 128),
                            _cap(xp[:], [(IMLEN, 128), (WP, 2), (1, CHUNKP)],
                                 offset=GUARD + (h0 + 3 + d0) * WP + 3),
                        ))
                    # odd tap (3,3) paired with the host-zeroed 50th tap slot
                    mms.append((
                        _cap(diag[:], [(50 * 128, 128), (128, 2), (1, 128)],
                             offset=48 * 128),
                        _cap(xp[:], [(IMLEN, 128), (0, 2), (1, CHUNKP)],
                             offset=GUARD + (h0 + 6) * WP + 3),
                    ))
                    for i, (wts, mv) in enumerate(mms):
                        nc.tensor.matmul(
                            ps[:], wts, mv,
                            start=(i == 0), stop=(i == len(mms) - 1), perf_mode=DR,
                        )
                    # evacuate: strip pad cols, add dw bias, fp8 out
                    nc.vector.tensor_scalar(
                        acc8[:, cb, sl].rearrange("p (h w) -> p h w", w=W),
                        ps[:].rearrange("p (h w) -> p h w", w=WP)[:, :, 3:59],
                        dwb_sb[:, cb : cb + 1],
                        None,
                        ALU.add,
                    )

            stats_out = {}

            def emit_stats(img, ch):
                sl = slice(ch * CHUNK, (ch + 1) * CHUNK)
                acc8 = get_acc(img)
                ysq = ysqp.tile([128, CB, CHUNK], FP8, tag="ysq")
                for cb in range(CB):
                    nc.scalar.activation(
                        ysq[:, cb, :], acc8[:, cb, sl], AF.Square, scale=SQS
                    )
                pmu = psstatp.tile([32, CHUNK], F32, tag="pmu", padded_shape=[32, 512])
                nc.tensor.matmul(
                    pmu[:], mw_full[:], acc8[:, 0:2, sl],
                    start=True, stop=False, perf_mode=DR,
                )
                nc.tensor.matmul(
                    pmu[:], mw_half[:],
                    acc8[:, 2, sl].unsqueeze(1).to_broadcast((128, 2, CHUNK)),
                    start=False, stop=True, perf_mode=DR,
                )
                pmsq = psstatp.tile([32, CHUNK], F32, tag="pmsq", padded_shape=[32, 512])
                nc.tensor.matmul(
                    pmsq[:], mw_full[:], ysq[:, 0:2, :],
                    start=True, stop=False, perf_mode=DR,
                )
                nc.tensor.matmul(
                    pmsq[:], mw_half[:],
                    ysq[:, 2, :].unsqueeze(1).to_broadcast((128, 2, CHUNK)),
                    start=False, stop=True, perf_mode=DR,
                )
                mu32 = smallp.tile([1, CHUNK], F32, tag="mu32")
                nc.vector.tensor_scalar_mul(mu32[:], pmu[0:1, :], 1.0 / C)
                tq = smallp.tile([1, CHUNK], F32, tag="tq")
                nc.vector.tensor_mul(tq[:], mu32[:], mu32[:])
                vchunk = smallp.tile([1, CHUNK], F32, tag="vchunk")
                nc.vector.scalar_tensor_tensor(
                    out=vchunk[:], in0=pmsq[0:1, :], scalar=(1.0 / SQS**2) / C,
                    in1=tq[:], op0=ALU.mult, op1=ALU.subtract,
                )
                nc.sync.dma_start(vscratch.ap()[img : img + 1, sl], vchunk[0:1, :])

                # Newton rsqrt in [56,8] transposed layout
                vpf = smallp.tile([56, 8], F32, tag="vpf")
                nc.sync.dma_start(
                    vpf[:], vscratch.ap()[img, sl].rearrange("(p f) -> p f", p=56)
                )
                v_eps = smallp.tile([56, 8], F32, tag="veps")
                nc.vector.tensor_scalar_add(v_eps[:], vpf[:], EPS_S)
                yr = smallp.tile([56, 8], F32, tag="yr")
                ti = smallp.tile([56, 8], I32, tag="ti")
                nc.vector.tensor_scalar(
                    ti[:], v_eps[:].bitcast(I32), 1, None, ALU.logical_shift_right
                )
                nc.vector.tensor_scalar(ti[:], ti[:], -1, None, ALU.bitwise_xor)
                nc.vector.tensor_scalar(
                    yr[:].bitcast(I32), ti[:], MAGIC + 1, None, ALU.add
                )
                rr = smallp.tile([56, 8], F32, tag="rr")
                yrb = smallp.tile([56, 8], BF16, tag="yrb")
                for it in range(2):
                    nc.vector.tensor_mul(rr[:], yr[:], yr[:])
                    nc.vector.tensor_mul(rr[:], rr[:], v_eps[:])
                    nc.vector.tensor_scalar(rr[:], rr[:], -0.5, 1.5, ALU.mult, ALU.add)
                    nc.vector.tensor_mul(yrb[:] if it == 1 else yr[:], yr[:], rr[:])
                nc.sync.dma_start(
                    rscratch.ap()[img, sl].rearrange("(p f) -> p f", p=56), yrb[:]
                )
                rb = rbp.tile([128, CHUNK], BF16, tag="rb")
                nc.sync.dma_start(
                    rb[:], rscratch.ap()[img, sl].partition_broadcast(128)
                )
                xts = xtsp.tile([128, 4, CHUNK], FP8, tag="xts")
                nc.gpsimd.memset(xts[:, 3, :], 0.0)
                nc.vector.tensor_mul(xts[0:1, 3, :], mu32[:], rb[0:1, :])
                for cb in range(CB):
                    nc.gpsimd.tensor_tensor(
                        xts[:, cb, :], acc8[:, cb, sl], rb[:], ALU.mult
                    )
                stats_out[(img, ch)] = xts

            def emit_mlp(img, ch):
                sl = slice(ch * CHUNK, (ch + 1) * CHUNK)
                xts = stats_out[(img, ch)]
                hblk = hp.tile([128, NFC, CHUNK], FP8, tag="h")
                for fc in range(NFC):
                    ps1 = ps1p.tile([128, CHUNK], F32, tag="p1", padded_shape=[128, 512])
                    nc.tensor.matmul(
                        ps1[:], w1_sb[:, 0:2, fc, :], xts[:, 0:2, :],
                        start=True, stop=False, perf_mode=DR,
                    )
                    nc.tensor.matmul(
                        ps1[:], w1_sb[:, 2:4, fc, :], xts[:, 2:4, :],
                        start=False, stop=True, perf_mode=DR,
                    )
                    nc.scalar.activation(
                        hblk[:, fc, :], ps1[:], AF.Gelu,
                        bias=b1_sb[:, fc : fc + 1], scale=1.0 / S_W1,
                    )
                for cb in range(CB):
                    cs = slice(cb * 128, (cb + 1) * 128)
                    ps2 = ps2p.tile([128, CHUNK], F32, tag="p2", padded_shape=[128, 512])
                    for k in range(6):
                        nc.tensor.matmul(
                            ps2[:], w2_sb[:, 2 * k : 2 * k + 2, cs],
                            hblk[:, 2 * k : 2 * k + 2, :],
                            start=(k == 0), stop=False, perf_mode=DR,
                        )
                    nc.tensor.matmul(
                        ps2[:], w2_sb[:, 12:14, cs],
                        ones8[:].unsqueeze(1).to_broadcast((128, 2, CHUNK)),
                        start=False, stop=True, perf_mode=DR,
                    )
                    xres = outp.tile([128, CHUNK], F32, tag="xres")
                    nc.sync.dma_start(xres[:], xs3[img, cs, sl])
                    osb = outp.tile([128, CHUNK], F32, tag="osb")
                    nc.vector.scalar_tensor_tensor(
                        out=osb[:], in0=ps2[:], scalar=g32_sb[:, cb : cb + 1],
                        in1=xres[:], op0=ALU.mult, op1=ALU.add,
                    )
                    nc.sync.dma_start(ys3[img, cs, sl], osb[:])

            # ---------------- pipeline ----------------
            # chunk stream across both images; conv runs 2 chunks ahead
            stream = [(img, ch) for img in range(IMGS_PER_CORE)
                      for ch in range(NCHUNK)]
            emit_xpad(0)
            emit_conv_chunk(*stream[0])
            emit_conv_chunk(*stream[1])
            for i, (img, ch) in enumerate(stream):
                emit_stats(img, ch)
                emit_mlp(img, ch)
                if i + 2 < len(stream):
                    nimg, nch = stream[i + 2]
                    if nimg == 1 and (1, 0) not in xpads:
                        emit_xpad(1)
                    emit_conv_chunk(nimg, nch)

    if split_waits:
        _split_waits(nc)
    return nc


_NC_CACHE = None


def _host_fold(inputs):
    f8 = ml_dtypes.float8_e4m3
    dw_w = np.asarray(inputs["dw_w"], dtype=np.float32)
    dw_b = np.asarray(inputs["dw_b"], dtype=np.float32)
    ln_w = np.asarray(inputs["ln_w"], dtype=np.float32)
    ln_b = np.asarray(inputs["ln_b"], dtype=np.float32)
    w1 = np.asarray(inputs["w1"], dtype=np.float32)
    b1 = np.asarray(inputs["b1"], dtype=np.float32)
    w2 = np.asarray(inputs["w2"], dtype=np.float32)
    b2 = np.asarray(inputs["b2"], dtype=np.float32)
    gamma = np.asarray(inputs["gamma"], dtype=np.float32)

    wtap = dw_w[:, :, 0, :].transpose(2, 0, 1).reshape(C, 49)  # [C, 49]
    wt8v = (S_DW * wtap).astype(f8).reshape(CB, 128, 49).transpose(1, 0, 2)
    diagv = np.zeros((128, CB, 50, 128), dtype=f8)
    ii = np.arange(128)
    diagv[ii, :, :49, ii] = wt8v

    dwb64v = np.ascontiguousarray((S_DW * dw_b).reshape(CB, 128).T)  # [128, CB]

    w1p = ln_w[:, None] * w1
    w1q = (S_W1 * w1p).astype(f8)                      # [C, FD]
    s1q = (-w1q.astype(np.float32).sum(axis=0)).astype(f8)  # [FD]
    w1arr = np.zeros((128, 4, NFC, 128), dtype=f8)
    w1arr[:, 0:3] = w1q.reshape(CB, 128, NFC, 128).transpose(1, 0, 2, 3)
    w1arr[0, 3] = s1q.reshape(NFC, 128)

    b1pv = np.ascontiguousarray(
        (b1 + ln_b @ w1).astype(np.float32).reshape(NFC, 128).T
    )  # [128, NFC]

    w2q = (S_W2 * w2).astype(f8)                       # [FD, C]
    w2arr = np.zeros((128, 14, C), dtype=f8)
    w2arr[:, 0:12] = w2q.reshape(NFC, 128, C).transpose(1, 0, 2)
    w2arr[0, 12] = (S_W2 * b2).astype(f8)

    g32v = np.ascontiguousarray((gamma / S_W2).astype(np.float32).reshape(CB, 128).T)

    return {
        "diag8": np.ascontiguousarray(diagv).reshape(128, CB * 50 * 128),
        "dwb64": dwb64v,
        "w1q8": np.ascontiguousarray(w1arr).reshape(128, 4 * NFC * 128),
        "b1p": b1pv,
        "w2q8": np.ascontiguousarray(w2arr).reshape(128, 14 * C),
        "g32": g32v,
    }


def _pad_fp8(xc):
    """[n, C, H, W] fp32 -> [n, CB, 128, IMLEN] fp8 padded image."""
    f8 = ml_dtypes.float8_e4m3
    n = xc.shape[0]
    xq = xc.astype(f8).reshape(n, CB, 128, H, W)
    grid = np.zeros((n, CB, 128, WP, WP), dtype=f8)
    grid[:, :, :, 3 : 3 + H, 3 : 3 + W] = xq
    out = np.zeros((n, CB, 128, IMLEN), dtype=f8)
    out[:, :, :, GUARD : GUARD + WP * WP] = grid.reshape(n, CB, 128, WP * WP)
    return out


def make_in_maps(inputs):
    x = np.asarray(inputs["x"], dtype=np.float32)
    common = _host_fold(inputs)
    in_maps = []
    for k in range(N_CORES):
        m = dict(common)
        xc = x[k * IMGS_PER_CORE : (k + 1) * IMGS_PER_CORE]
        m["xs"] = np.ascontiguousarray(xc)
        m["xp8"] = _pad_fp8(xc)
        in_maps.append(m)
    return in_maps


def kernel(**inputs):
    global _NC_CACHE
    in_maps = make_in_maps(inputs)
    if _NC_CACHE is None:
        _NC_CACHE = _build_nc()
    res = run_bass_kernel_spmd(_NC_CACHE, in_maps, core_ids=list(range(N_CORES)))
    out = np.concatenate([res.results[k]["ys"] for k in range(N_CORES)], axis=0)
    return out.astype(np.float32)


if __name__ == "__main__":
    rng = np.random.default_rng(0)
    ins = {
        "x": rng.standard_normal((16, C, H, W), dtype=np.float32),
        "dw_w": 0.02 * rng.standard_normal((7, 7, 1, C), dtype=np.float32),
        "dw_b": 0.02 * rng.standard_normal((C,), dtype=np.float32),
        "ln_w": np.ones(C, np.float32),
        "ln_b": np.zeros(C, np.float32),
        "w1": (C**-0.5) * rng.standard_normal((C, FD), dtype=np.float32),
        "b1": 0.02 * rng.standard_normal((FD,), dtype=np.float32),
        "w2": ((4 * C) ** -0.5) * rng.standard_normal((FD, C), dtype=np.float32),
        "b2": 0.02 * rng.standard_normal((C,), dtype=np.float32),
        "gamma": np.full((C,), 1e-6, np.float32),
    }
    out = kernel(**ins)
    print("out", out.shape, out.dtype, np.abs(out).mean())
